# revision 2
# baseline (speedup 1.0000x reference)
"""Malvar-He-Cutler demosaic as a Trainium2 Bass kernel.

Strategy (per core; batch 16 is sharded 2 images/core across 8 cores):
  - All 12 output planes (3 channels x 2x2 Bayer quadrant) are quarter-res
    images.  8 are 5x5 convs of the padded mosaic, 4 are identity copies.
    Every plane (identity included) is computed as a short sum of
    banded-matrix matmuls on the TensorEngine: contraction runs over image
    quad-rows held in SBUF partitions, column taps become stride-2
    access-pattern offsets of the moving operand.
  - One-pass fp16: x is cast to fp16 during the load DMA (SWDGE cast) and
    the 5x5 coefficients (multiples of 1/16) are exact in fp16; fp32 PSUM
    accumulation keeps the max relative error ~3e-4, well inside 2e-2.
  - Row-pair SBUF layout: partition p holds both image rows of quad-row
    (i0-1)+p, so each block needs ONE contiguous 1 MiB load (128 x 8 KiB
    descriptors) instead of strided row gathers.  Outputs are staged as
    [quad-row, row-parity, col] so each channel's block store is a single
    contiguous 1 MiB DMA.
  - DMA dispatch is spread over three queues: loads on SWDGE (GpSimd),
    stores alternating between the SP and Activation HWDGE queues.
"""

import os
import sys

import numpy as np

for _p in ("/opt/trn_rl_repo", "/root/.axon_site/_ro/trn_rl_repo"):
    if os.path.isdir(_p) and _p not in sys.path:
        sys.path.insert(0, _p)

import concourse.bacc as bacc
import concourse.mybir as mybir
import concourse.tile as tile

# ---------------------------------------------------------------- constants
_K = [
    0, 0, -2, 0, 0,  0, 0, 4, 0, 0,  -2, 4, 8, 4, -2,  0, 0, 4, 0, 0,  0, 0, -2, 0, 0,
    0, 0, -3, 0, 0,  0, 4, 0, 4, 0,  -3, 0, 12, 0, -3,  0, 4, 0, 4, 0,  0, 0, -3, 0, 0,
    0, 0, 1, 0, 0,  0, -2, 0, -2, 0,  -2, 8, 10, 8, -2,  0, -2, 0, -2, 0,  0, 0, 1, 0, 0,
    0, 0, -2, 0, 0,  0, -2, 8, -2, 0,  1, 0, 10, 0, 1,  0, -2, 8, -2, 0,  0, 0, -2, 0, 0,
]
KER = np.asarray(_K, dtype=np.float64).reshape(4, 5, 5) / 16.0
INDICES_RGGB = np.array([4, 2, 3, 1, 0, 4, 4, 0, 1, 3, 2, 4]).reshape(1, 3, 2, 2)

H = W = 1024
QH = H // 2          # quad rows per image
QW = W // 2          # quad cols per image
IMGS_PER_CORE = 2
N_CORES = 8
MBLK = 126           # output quad rows per full block
KPART = 128          # contraction partitions: p <-> quad row (i0-1)+p
EW = W + 4           # halo-padded row width


def _calc_index(pattern):
    p = tuple(np.asarray(pattern).flatten().tolist())
    if p == (0, 1, 1, 2):
        return INDICES_RGGB
    if p == (1, 0, 2, 1):
        return np.roll(INDICES_RGGB, 1, axis=-1)
    if p == (1, 2, 0, 1):
        return np.roll(INDICES_RGGB, 1, axis=-2)
    if p == (2, 1, 1, 0):
        return np.roll(np.roll(INDICES_RGGB, 1, axis=-1), 1, axis=-2)
    raise ValueError("Invalid bayer pattern")


def _matmul_groups(k, a, b):
    """Group the nonzero taps of kernel k for output quadrant (a, b) by
    (source row-parity q, column offset).  Each group is one banded
    matmul; tap quad-row offset d maps to partition p = mm + 1 + d."""
    groups = {}
    for dy in range(-2, 3):
        for dx in range(-2, 3):
            c = KER[k, dy + 2, dx + 2]
            if c == 0.0:
                continue
            q = (a + dy) % 2
            d = (a + dy - q) // 2
            coff = b + dx + 2
            bands = groups.setdefault((q, coff), {})
            bands[d] = bands.get(d, 0.0) + c
    return groups


def _bmat(bands):
    B = np.zeros((KPART, MBLK), np.float32)
    for mm in range(MBLK):
        for d, c in bands.items():
            B[mm + 1 + d, mm] = c
    return B


def _build_plan(index):
    """index: (3,2,2).  Returns (planes, bmats); planes is a list of
    (c, a, b, glist) where glist = [(bmat_idx, q, coff), ...]; identity
    planes become single-band matmuls (band {0: 1.0})."""
    bmats = []
    bkey = {}

    def intern(bands):
        key = tuple(sorted((d, round(v * 16)) for d, v in bands.items()))
        if key not in bkey:
            bkey[key] = len(bmats)
            bmats.append(_bmat(bands))
        return bkey[key]

    planes = []
    for c in range(3):
        for a in range(2):
            for b in range(2):
                k = int(index[c, a, b])
                if k == 4:
                    glist = [(intern({0: 1.0}), a, b + 2)]
                else:
                    glist = [
                        (intern(bands), q, coff)
                        for (q, coff), bands in sorted(
                            _matmul_groups(k, a, b).items()
                        )
                    ]
                planes.append((c, a, b, glist))
    return planes, np.stack(bmats)


# ------------------------------------------------------------ bass program
def build_nc(planes, n_bmats, reps=1):
    f32, f16 = mybir.dt.float32, mybir.dt.float16
    nc = bacc.Bacc("TRN2", target_bir_lowering=False, debug=False)
    x_d = nc.dram_tensor("x", [IMGS_PER_CORE, QH, 2, W], f32, kind="ExternalInput")
    bm_d = nc.dram_tensor("bm", [KPART, n_bmats, MBLK], f16, kind="ExternalInput")
    y_d = nc.dram_tensor(
        "y", [IMGS_PER_CORE, 3, QH, 2, W], f32, kind="ExternalOutput"
    )

    i0s = list(range(0, QH, MBLK))  # block starts

    with tile.TileContext(nc) as tc:
        with (
            tc.tile_pool(name="consts", bufs=1) as cpool,
            tc.tile_pool(name="esrc", bufs=3) as epool,
            tc.tile_pool(name="stage", bufs=2) as spool,
            tc.tile_pool(name="psum", bufs=8, space="PSUM") as ppool,
        ):
            bw = cpool.tile([KPART, n_bmats, MBLK], f16, tag="bw", name="bw")
            nc.sync.dma_start(bw[:, :, :], bm_d[:, :, :])

            for rep in range(reps):
                for img in range(IMGS_PER_CORE):
                    for bi, i0 in enumerate(i0s):
                        m = min(MBLK, QH - i0)   # output quad rows this block
                        kblk = m + 2             # contraction partitions used

                        # ordered load: partition p <-> quad row (i0-1)+p,
                        # both row parities side by side (fp16 cast in DMA)
                        e = epool.tile([KPART, 2, EW], f16, tag="e",
                                       name=f"e_{img}_{bi}")
                        p_lo = 1 if i0 == 0 else 0
                        p_hi = min(kblk, QH + 1 - i0)  # quad rows < QH
                        nc.gpsimd.dma_start(
                            e[p_lo:p_hi, :, 2 : 2 + W],
                            x_d[img, i0 - 1 + p_lo : i0 - 1 + p_hi, :, :],
                        )
                        if i0 == 0:  # halo above: quad -1 -> row 0 twice
                            for q in range(2):
                                nc.gpsimd.dma_start(
                                    e[0:1, q, 2 : 2 + W],
                                    x_d[img, 0:1, 0:1, :],
                                )
                        if p_hi < kblk:  # halo below: quad QH -> row H-1 twice
                            for q in range(2):
                                nc.gpsimd.dma_start(
                                    e[p_hi : p_hi + 1, q, 2 : 2 + W],
                                    x_d[img, QH - 1 : QH, 1:2, :],
                                )
                        # horizontal replication pad (tiny DVE copies)
                        for q in range(2):
                            nc.vector.tensor_copy(e[:, q, 0:1], e[:, q, 2:3])
                            nc.vector.tensor_copy(e[:, q, 1:2], e[:, q, 2:3])
                            nc.vector.tensor_copy(
                                e[:, q, W + 2 : W + 3], e[:, q, W + 1 : W + 2]
                            )
                            nc.vector.tensor_copy(
                                e[:, q, W + 3 : W + 4], e[:, q, W + 1 : W + 2]
                            )

                        stg = {}
                        for c in range(3):
                            stg[c] = spool.tile([MBLK, 2, W], f32, tag=f"st{c}",
                                                name=f"st{c}_{img}_{bi}")

                        for pi, (c, a, b, glist) in enumerate(planes):
                            ps = ppool.tile([MBLK, QW], f32, tag="ps",
                                            name=f"ps{c}{a}{b}_{img}_{bi}")
                            nmm = len(glist)
                            for i_mm, (bmi, q, coff) in enumerate(glist):
                                nc.tensor.matmul(
                                    ps[0:m, :],
                                    bw[0:kblk, bmi, 0:m],
                                    e[0:kblk, q, coff : coff + W - 1 : 2],
                                    start=(i_mm == 0),
                                    stop=(i_mm == nmm - 1),
                                )
                            dst = stg[c][0:m, a, b : b + W - 1 : 2]
                            if pi % 2 == 0:
                                nc.vector.tensor_copy(dst, ps[0:m, :])
                            else:
                                nc.scalar.copy(dst, ps[0:m, :])

                        for c in range(3):
                            eng = nc.sync if c % 2 == 0 else nc.scalar
                            eng.dma_start(
                                y_d[img, c, i0 : i0 + m, :, :],
                                stg[c][0:m, :, :],
                            )
    nc.compile()
    return nc


# ------------------------------------------------------------- SPMD runner
_CACHE = {}


def _get_compiled(index_key, planes, n_bmats, reps=1):
    key = (index_key, reps)
    if key not in _CACHE:
        _CACHE[key] = build_nc(planes, n_bmats, reps=reps)
    return _CACHE[key]


_RUNNER_CACHE = {}


def make_runner(nc, n_cores=N_CORES):
    """Cached jitted SPMD executor mirroring bass2jax.run_bass_via_pjrt's
    multi-core path, reusable across calls without re-tracing."""
    import jax
    import concourse.mybir as mybir_
    from concourse import bass2jax
    from jax.experimental.shard_map import shard_map
    from jax.sharding import Mesh, PartitionSpec

    bass2jax.install_neuronx_cc_hook()

    partition_name = (
        nc.partition_id_tensor.name if nc.partition_id_tensor else None
    )
    in_names, out_names, out_avals, zero_outs = [], [], [], []
    for alloc in nc.m.functions[0].allocations:
        if not isinstance(alloc, mybir_.MemoryLocationSet):
            continue
        name = alloc.memorylocations[0].name
        if alloc.kind == "ExternalInput":
            if name != partition_name:
                in_names.append(name)
        elif alloc.kind == "ExternalOutput":
            shape = tuple(alloc.tensor_shape)
            dtype = mybir_.dt.np(alloc.dtype)
            out_names.append(name)
            out_avals.append(jax.core.ShapedArray(shape, dtype))
            zero_outs.append(np.zeros(shape, dtype))
    n_params = len(in_names)
    n_outs = len(out_avals)
    all_in_names = in_names + out_names
    if partition_name is not None:
        all_in_names.append(partition_name)

    def _body(*args):
        operands = list(args)
        if partition_name is not None:
            operands.append(bass2jax.partition_id_tensor())
        outs = bass2jax._bass_exec_p.bind(
            *operands,
            out_avals=tuple(out_avals),
            in_names=tuple(all_in_names),
            out_names=tuple(out_names),
            lowering_input_output_aliases=(),
            sim_require_finite=True,
            sim_require_nnan=True,
            nc=nc,
        )
        return tuple(outs)

    devices = jax.devices()[:n_cores]
    mesh = Mesh(np.asarray(devices), ("core",))
    sharded = jax.jit(
        shard_map(
            _body, mesh=mesh,
            in_specs=(PartitionSpec("core"),) * (n_params + n_outs),
            out_specs=(PartitionSpec("core"),) * n_outs,
            check_rep=False,
        ),
        donate_argnums=tuple(range(n_params, n_params + n_outs)),
        keep_unused=True,
    )

    def run(in_maps):
        concat_in = [
            np.concatenate([np.asarray(m[name]) for m in in_maps], axis=0)
            for name in in_names
        ]
        concat_zeros = [
            np.zeros((n_cores * z.shape[0], *z.shape[1:]), z.dtype)
            for z in zero_outs
        ]
        out_arrs = sharded(*concat_in, *concat_zeros)
        return [
            {
                name: np.asarray(out_arrs[i]).reshape(
                    n_cores, *out_avals[i].shape
                )[c]
                for i, name in enumerate(out_names)
            }
            for c in range(n_cores)
        ]

    return run


def get_runner(reps=1, index=None, **_ignored):
    if index is None:
        index = INDICES_RGGB
    index3 = np.asarray(index).reshape(3, 2, 2)
    ikey = tuple(index3.flatten().tolist())
    key = (ikey, reps)
    if key not in _RUNNER_CACHE:
        planes, bmats = _build_plan(index3)
        nc = _get_compiled(ikey, planes, len(bmats), reps=reps)
        run = make_runner(nc)
        _RUNNER_CACHE[key] = (run, bmats)
    return _RUNNER_CACHE[key]


def pack_bmats(bmats):
    # [nb, KPART, MBLK] -> [KPART, nb, MBLK] contiguous fp16
    return np.ascontiguousarray(
        np.transpose(bmats, (1, 0, 2)).astype(np.float16)
    )


def make_inmaps(x, bmats):
    n = x.shape[0]
    xs = x.reshape(n, QH, 2, W)
    per = n // N_CORES
    bm = pack_bmats(bmats)
    return [
        {"x": xs[c * per : (c + 1) * per], "bm": bm}
        for c in range(N_CORES)
    ]


def kernel(x, bayer_pattern):
    x = np.ascontiguousarray(np.asarray(x), dtype=np.float32)
    bp = np.asarray(bayer_pattern)
    assert bp.reshape(-1, 4).shape[0] == 1, "per-batch bayer patterns unsupported"
    index = _calc_index(bp)
    run, bmats = get_runner(index=index)
    results = run(make_inmaps(x.reshape(x.shape[0], H, W), bmats))
    out = np.concatenate(
        [r["y"].reshape(IMGS_PER_CORE, 3, H, W) for r in results], axis=0
    )
    return out.astype(np.float32)


# revision 12
# speedup vs baseline: 1.5112x; 1.5112x over previous
"""Malvar-He-Cutler demosaic as a Trainium2 Bass kernel.

Strategy (per core; batch 16 is sharded 2 images/core across 8 cores):
  - All 12 output planes (3 channels x 2x2 Bayer quadrant) are quarter-res
    images.  8 are 5x5 convs of the padded mosaic, 4 are identity copies.
    Every plane (identity included) is computed as a short sum of
    banded-matrix matmuls on the TensorEngine: contraction runs over image
    quad-rows held in SBUF partitions, column taps become stride-2
    access-pattern offsets of the moving operand.
  - One-pass fp16: x is cast to fp16 during the load DMA (SWDGE cast) and
    the 5x5 coefficients (multiples of 1/16) are exact in fp16; fp32 PSUM
    accumulation keeps the max relative error ~3e-4, well inside 2e-2.
  - Row-pair SBUF layout: partition p holds both image rows of quad-row
    (i0-1)+p, so each block needs ONE contiguous 1 MiB load (128 x 8 KiB
    descriptors) instead of strided row gathers.  Outputs are staged as
    [quad-row, row-parity, col] so each channel's block store is a single
    contiguous 1 MiB DMA.
  - DMA dispatch is spread over three queues: loads on SWDGE (GpSimd),
    stores alternating between the SP and Activation HWDGE queues.
"""

import os
import sys

import numpy as np

for _p in ("/opt/trn_rl_repo", "/root/.axon_site/_ro/trn_rl_repo"):
    if os.path.isdir(_p) and _p not in sys.path:
        sys.path.insert(0, _p)

import concourse.bacc as bacc
import concourse.mybir as mybir
import concourse.tile as tile

# ---------------------------------------------------------------- constants
_K = [
    0, 0, -2, 0, 0,  0, 0, 4, 0, 0,  -2, 4, 8, 4, -2,  0, 0, 4, 0, 0,  0, 0, -2, 0, 0,
    0, 0, -3, 0, 0,  0, 4, 0, 4, 0,  -3, 0, 12, 0, -3,  0, 4, 0, 4, 0,  0, 0, -3, 0, 0,
    0, 0, 1, 0, 0,  0, -2, 0, -2, 0,  -2, 8, 10, 8, -2,  0, -2, 0, -2, 0,  0, 0, 1, 0, 0,
    0, 0, -2, 0, 0,  0, -2, 8, -2, 0,  1, 0, 10, 0, 1,  0, -2, 8, -2, 0,  0, 0, -2, 0, 0,
]
KER = np.asarray(_K, dtype=np.float64).reshape(4, 5, 5) / 16.0
INDICES_RGGB = np.array([4, 2, 3, 1, 0, 4, 4, 0, 1, 3, 2, 4]).reshape(1, 3, 2, 2)

H = W = 1024
QH = H // 2          # quad rows per image
QW = W // 2          # quad cols per image
IMGS_PER_CORE = 2
N_CORES = 8
MBLK = 126           # output quad rows per full block
KPART = 128          # contraction partitions: p <-> quad row (i0-1)+p
EW = W + 4           # halo-padded row width


def _calc_index(pattern):
    p = tuple(np.asarray(pattern).flatten().tolist())
    if p == (0, 1, 1, 2):
        return INDICES_RGGB
    if p == (1, 0, 2, 1):
        return np.roll(INDICES_RGGB, 1, axis=-1)
    if p == (1, 2, 0, 1):
        return np.roll(INDICES_RGGB, 1, axis=-2)
    if p == (2, 1, 1, 0):
        return np.roll(np.roll(INDICES_RGGB, 1, axis=-1), 1, axis=-2)
    raise ValueError("Invalid bayer pattern")


def _matmul_groups(k, a, b):
    """Group the nonzero taps of kernel k for output quadrant (a, b) by
    (source row-parity q, column offset).  Each group is one banded
    matmul; tap quad-row offset d maps to partition p = mm + 1 + d."""
    groups = {}
    for dy in range(-2, 3):
        for dx in range(-2, 3):
            c = KER[k, dy + 2, dx + 2]
            if c == 0.0:
                continue
            q = (a + dy) % 2
            d = (a + dy - q) // 2
            coff = b + dx + 2
            bands = groups.setdefault((q, coff), {})
            bands[d] = bands.get(d, 0.0) + c
    return groups


def _bmat(bands):
    B = np.zeros((KPART, MBLK), np.float32)
    for mm in range(MBLK):
        for d, c in bands.items():
            B[mm + 1 + d, mm] = c
    return B


TAILM = QH - 4 * MBLK       # quad rows in the per-image tail (8)
TAILP1 = TAILM + 2          # img1 tail partition base in the merged block


def _bmat_tail(bands):
    """Merged-tail band matrix: outputs 0..TAILM-1 are image 0's last quad
    rows (partitions mm+1+d), outputs TAILM..2*TAILM-1 are image 1's
    (partitions TAILP1 + (mm-TAILM) + 1 + d)."""
    B = np.zeros((KPART, MBLK), np.float32)
    for mm in range(2 * TAILM):
        base = (mm + 1) if mm < TAILM else (TAILP1 + mm - TAILM + 1)
        for d, c in bands.items():
            B[base + d, mm] = c
    return B


def _build_plan(index):
    """index: (3,2,2).  Returns (planes, bmats); planes is a list of
    (c, a, b, glist) where glist = [(bmat_idx, q, coff), ...]; identity
    planes become single-band matmuls (band {0: 1.0}).  For every band
    structure two matrices are interned: the main-block one and the
    merged-tail one (index + n_struct)."""
    bmain = []
    btail = []
    bkey = {}

    def intern(bands):
        key = tuple(sorted((d, round(v * 16)) for d, v in bands.items()))
        if key not in bkey:
            bkey[key] = len(bmain)
            bmain.append(_bmat(bands))
            btail.append(_bmat_tail(bands))
        return bkey[key]

    planes = []
    for c in range(3):
        for a in range(2):
            for b in range(2):
                k = int(index[c, a, b])
                if k == 4:
                    glist = [(intern({0: 1.0}), a, b + 2)]
                else:
                    glist = [
                        (intern(bands), q, coff)
                        for (q, coff), bands in sorted(
                            _matmul_groups(k, a, b).items()
                        )
                    ]
                planes.append((c, a, b, glist))
    return planes, np.stack(bmain + btail)


# ------------------------------------------------------------ bass program
def build_nc(planes, n_bmats, reps=1):
    f32, f16 = mybir.dt.float32, mybir.dt.float16
    nc = bacc.Bacc("TRN2", target_bir_lowering=False, debug=False)
    x_d = nc.dram_tensor("x", [IMGS_PER_CORE, QH, 2, W], f16, kind="ExternalInput")
    bm_d = nc.dram_tensor("bm", [KPART, n_bmats, MBLK], f16, kind="ExternalInput")
    y_d = nc.dram_tensor(
        "y", [IMGS_PER_CORE, 3, QH, 2, W], f32, kind="ExternalOutput"
    )

    n_struct = n_bmats // 2  # main matrices; tail copies live at +n_struct
    i0s = [i * MBLK for i in range(QH // MBLK)]  # full main blocks
    i0_tail = len(i0s) * MBLK

    with tile.TileContext(nc) as tc:
        with (
            tc.tile_pool(name="consts", bufs=1) as cpool,
            tc.tile_pool(name="esrc", bufs=3) as epool,
            tc.tile_pool(name="stage", bufs=3) as spool,
            tc.tile_pool(name="psum", bufs=8, space="PSUM") as ppool,
        ):
            bw = cpool.tile([KPART, n_bmats, MBLK], f16, tag="bw", name="bw")
            nc.scalar.dma_start(bw[:, :, :], bm_d[:, :, :])

            for rep in range(reps):
                # blocks: (kind, img, i0); merged tail handles both images
                blocks = [
                    ("main", img, i0)
                    for img in range(IMGS_PER_CORE)
                    for i0 in i0s
                ] + [("tail", 0, i0_tail)]

                for gbi, (kind, img, i0) in enumerate(blocks):
                    if kind == "main":
                        m, kblk = MBLK, MBLK + 2
                    else:
                        m, kblk = 2 * TAILM, TAILP1 + TAILM + 2

                    er = epool.tile([KPART, 2, W], f16, tag="er",
                                    name=f"er_{gbi}")
                    ld = nc.sync if gbi % 2 == 0 else nc.scalar
                    if kind == "main":
                        p_lo = 1 if i0 == 0 else 0
                        if i0 == 0:  # halo above: quad -1 -> row 0 twice
                            for q in range(2):
                                ld.dma_start(
                                    er[0:1, q, :], x_d[img, 0:1, 0:1, :]
                                )
                        ld.dma_start(
                            er[p_lo : kblk, :, :],
                            x_d[img, i0 - 1 + p_lo : i0 - 1 + kblk, :, :],
                        )
                    else:
                        # image 0 tail at partitions 0..TAILM, image 1 tail
                        # at TAILP1..TAILP1+TAILM; below-halo partitions
                        # clamp to each image's last row.
                        for im in range(IMGS_PER_CORE):
                            pb = im * TAILP1
                            for q in range(2):
                                ld.dma_start(
                                    er[pb + TAILM + 1 : pb + TAILM + 2, q, :],
                                    x_d[im, QH - 1 : QH, 1:2, :],
                                )
                            ld.dma_start(
                                er[pb : pb + TAILM + 1, :, :],
                                x_d[im, i0 - 1 : QH, :, :],
                            )
                    e = epool.tile([KPART, 2, EW], f16, tag="e",
                                   name=f"e_{gbi}")
                    for q in range(2):
                        nc.vector.tensor_copy(
                            e[0:kblk, q, 2 : 2 + W], er[0:kblk, q, :]
                        )
                    # horizontal replication pad (tiny copies on idle GpSimd)
                    for q in range(2):
                        nc.gpsimd.tensor_copy(e[:, q, 0:1], e[:, q, 2:3])
                        nc.gpsimd.tensor_copy(e[:, q, 1:2], e[:, q, 2:3])
                        nc.gpsimd.tensor_copy(
                            e[:, q, W + 2 : W + 3], e[:, q, W + 1 : W + 2]
                        )
                        nc.gpsimd.tensor_copy(
                            e[:, q, W + 3 : W + 4], e[:, q, W + 1 : W + 2]
                        )

                    stg = {}
                    for c in range(3):
                        stg[c] = spool.tile([MBLK, 2, W], f32, tag=f"st{c}",
                                            name=f"st{c}_{gbi}")

                    boff = 0 if kind == "main" else n_struct
                    for pi, (c, a, b, glist) in enumerate(planes):
                        ps = ppool.tile([MBLK, QW], f32, tag="ps",
                                        name=f"ps{c}{a}{b}_{gbi}")
                        nmm = len(glist)
                        for i_mm, (bmi, q, coff) in enumerate(glist):
                            nc.tensor.matmul(
                                ps[0:m, :],
                                bw[0:kblk, boff + bmi, 0:m],
                                e[0:kblk, q, coff : coff + W - 1 : 2],
                                start=(i_mm == 0),
                                stop=(i_mm == nmm - 1),
                            )
                        dst = stg[c][0:m, a, b : b + W - 1 : 2]
                        if pi % 2 == 0:
                            nc.vector.tensor_copy(dst, ps[0:m, :])
                        else:
                            nc.scalar.copy(dst, ps[0:m, :])

                    for c in range(3):
                        eng = nc.sync if (gbi + c) % 2 == 1 else nc.scalar
                        if kind == "main":
                            eng.dma_start(
                                y_d[img, c, i0 : i0 + m, :, :],
                                stg[c][0:m, :, :],
                            )
                        else:
                            for im in range(IMGS_PER_CORE):
                                eng.dma_start(
                                    y_d[im, c, i0:QH, :, :],
                                    stg[c][im * TAILM : (im + 1) * TAILM, :, :],
                                )
    nc.compile()
    return nc


# ------------------------------------------------------------- SPMD runner
_CACHE = {}


def _get_compiled(index_key, planes, n_bmats, reps=1):
    key = (index_key, reps)
    if key not in _CACHE:
        _CACHE[key] = build_nc(planes, n_bmats, reps=reps)
    return _CACHE[key]


_RUNNER_CACHE = {}


def make_runner(nc, n_cores=N_CORES):
    """Cached jitted SPMD executor mirroring bass2jax.run_bass_via_pjrt's
    multi-core path, reusable across calls without re-tracing."""
    import jax
    import concourse.mybir as mybir_
    from concourse import bass2jax
    from jax.experimental.shard_map import shard_map
    from jax.sharding import Mesh, PartitionSpec

    bass2jax.install_neuronx_cc_hook()

    partition_name = (
        nc.partition_id_tensor.name if nc.partition_id_tensor else None
    )
    in_names, out_names, out_avals, zero_outs = [], [], [], []
    for alloc in nc.m.functions[0].allocations:
        if not isinstance(alloc, mybir_.MemoryLocationSet):
            continue
        name = alloc.memorylocations[0].name
        if alloc.kind == "ExternalInput":
            if name != partition_name:
                in_names.append(name)
        elif alloc.kind == "ExternalOutput":
            shape = tuple(alloc.tensor_shape)
            dtype = mybir_.dt.np(alloc.dtype)
            out_names.append(name)
            out_avals.append(jax.core.ShapedArray(shape, dtype))
            zero_outs.append(np.zeros(shape, dtype))
    n_params = len(in_names)
    n_outs = len(out_avals)
    all_in_names = in_names + out_names
    if partition_name is not None:
        all_in_names.append(partition_name)

    def _body(*args):
        operands = list(args)
        if partition_name is not None:
            operands.append(bass2jax.partition_id_tensor())
        outs = bass2jax._bass_exec_p.bind(
            *operands,
            out_avals=tuple(out_avals),
            in_names=tuple(all_in_names),
            out_names=tuple(out_names),
            lowering_input_output_aliases=(),
            sim_require_finite=True,
            sim_require_nnan=True,
            nc=nc,
        )
        return tuple(outs)

    devices = jax.devices()[:n_cores]
    mesh = Mesh(np.asarray(devices), ("core",))
    sharded = jax.jit(
        shard_map(
            _body, mesh=mesh,
            in_specs=(PartitionSpec("core"),) * (n_params + n_outs),
            out_specs=(PartitionSpec("core"),) * n_outs,
            check_rep=False,
        ),
        donate_argnums=tuple(range(n_params, n_params + n_outs)),
        keep_unused=True,
    )

    def run(in_maps):
        concat_in = [
            np.concatenate([np.asarray(m[name]) for m in in_maps], axis=0)
            for name in in_names
        ]
        concat_zeros = [
            np.zeros((n_cores * z.shape[0], *z.shape[1:]), z.dtype)
            for z in zero_outs
        ]
        out_arrs = sharded(*concat_in, *concat_zeros)
        return [
            {
                name: np.asarray(out_arrs[i]).reshape(
                    n_cores, *out_avals[i].shape
                )[c]
                for i, name in enumerate(out_names)
            }
            for c in range(n_cores)
        ]

    return run


def get_runner(reps=1, index=None, **_ignored):
    if index is None:
        index = INDICES_RGGB
    index3 = np.asarray(index).reshape(3, 2, 2)
    ikey = tuple(index3.flatten().tolist())
    key = (ikey, reps)
    if key not in _RUNNER_CACHE:
        planes, bmats = _build_plan(index3)
        nc = _get_compiled(ikey, planes, len(bmats), reps=reps)
        run = make_runner(nc)
        _RUNNER_CACHE[key] = (run, bmats)
    return _RUNNER_CACHE[key]


def pack_bmats(bmats):
    # [nb, KPART, MBLK] -> [KPART, nb, MBLK] contiguous fp16
    return np.ascontiguousarray(
        np.transpose(bmats, (1, 0, 2)).astype(np.float16)
    )


def make_inmaps(x, bmats):
    n = x.shape[0]
    xs = np.ascontiguousarray(x.reshape(n, QH, 2, W), dtype=np.float16)
    per = n // N_CORES
    bm = pack_bmats(bmats)
    return [
        {"x": xs[c * per : (c + 1) * per], "bm": bm}
        for c in range(N_CORES)
    ]


def kernel(x, bayer_pattern):
    x = np.ascontiguousarray(np.asarray(x), dtype=np.float32)
    bp = np.asarray(bayer_pattern)
    assert bp.reshape(-1, 4).shape[0] == 1, "per-batch bayer patterns unsupported"
    index = _calc_index(bp)
    run, bmats = get_runner(index=index)
    results = run(make_inmaps(x.reshape(x.shape[0], H, W), bmats))
    out = np.concatenate(
        [r["y"].reshape(IMGS_PER_CORE, 3, H, W) for r in results], axis=0
    )
    return out.astype(np.float32)


# revision 39
# speedup vs baseline: 1.8240x; 1.2070x over previous
"""Malvar-He-Cutler demosaic as a Trainium2 Bass kernel.

Strategy (per core; batch 16 is sharded 2 images/core across 8 cores):
  - All 12 output planes (3 channels x 2x2 Bayer quadrant) are quarter-res
    images.  8 are 5x5 convs of the padded mosaic, 4 are identity copies.
    Every plane (identity included) is computed as a short sum of
    banded-matrix matmuls on the TensorEngine: contraction runs over image
    quad-rows held in SBUF partitions, column taps become stride-2
    access-pattern offsets of the moving operand.
  - One-pass fp16: x is cast to fp16 during the load DMA (SWDGE cast) and
    the 5x5 coefficients (multiples of 1/16) are exact in fp16; fp32 PSUM
    accumulation keeps the max relative error ~3e-4, well inside 2e-2.
  - Row-pair SBUF layout: partition p holds both image rows of quad-row
    (i0-1)+p, so each block needs ONE contiguous 1 MiB load (128 x 8 KiB
    descriptors) instead of strided row gathers.  Outputs are staged as
    [quad-row, row-parity, col] so each channel's block store is a single
    contiguous 1 MiB DMA.
  - DMA dispatch is spread over three queues: loads on SWDGE (GpSimd),
    stores alternating between the SP and Activation HWDGE queues.
"""

import os
import sys

import numpy as np

for _p in ("/opt/trn_rl_repo", "/root/.axon_site/_ro/trn_rl_repo"):
    if os.path.isdir(_p) and _p not in sys.path:
        sys.path.insert(0, _p)

import concourse.bacc as bacc
import concourse.mybir as mybir
import concourse.tile as tile

# ---------------------------------------------------------------- constants
_K = [
    0, 0, -2, 0, 0,  0, 0, 4, 0, 0,  -2, 4, 8, 4, -2,  0, 0, 4, 0, 0,  0, 0, -2, 0, 0,
    0, 0, -3, 0, 0,  0, 4, 0, 4, 0,  -3, 0, 12, 0, -3,  0, 4, 0, 4, 0,  0, 0, -3, 0, 0,
    0, 0, 1, 0, 0,  0, -2, 0, -2, 0,  -2, 8, 10, 8, -2,  0, -2, 0, -2, 0,  0, 0, 1, 0, 0,
    0, 0, -2, 0, 0,  0, -2, 8, -2, 0,  1, 0, 10, 0, 1,  0, -2, 8, -2, 0,  0, 0, -2, 0, 0,
]
KER = np.asarray(_K, dtype=np.float64).reshape(4, 5, 5) / 16.0
INDICES_RGGB = np.array([4, 2, 3, 1, 0, 4, 4, 0, 1, 3, 2, 4]).reshape(1, 3, 2, 2)

H = W = 1024
QH = H // 2          # quad rows per image
QW = W // 2          # quad cols per image
IMGS_PER_CORE = 2
N_CORES = 8
MBLK = 126           # output quad rows per full block
KPART = 128          # contraction partitions: p <-> quad row (i0-1)+p
EW = W + 4           # halo-padded row width


def _calc_index(pattern):
    p = tuple(np.asarray(pattern).flatten().tolist())
    if p == (0, 1, 1, 2):
        return INDICES_RGGB
    if p == (1, 0, 2, 1):
        return np.roll(INDICES_RGGB, 1, axis=-1)
    if p == (1, 2, 0, 1):
        return np.roll(INDICES_RGGB, 1, axis=-2)
    if p == (2, 1, 1, 0):
        return np.roll(np.roll(INDICES_RGGB, 1, axis=-1), 1, axis=-2)
    raise ValueError("Invalid bayer pattern")


def _matmul_groups(k, a, b):
    """Group the nonzero taps of kernel k for output quadrant (a, b) by
    (source row-parity q, column offset).  Each group is one banded
    matmul; tap quad-row offset d maps to partition p = mm + 1 + d."""
    groups = {}
    for dy in range(-2, 3):
        for dx in range(-2, 3):
            c = KER[k, dy + 2, dx + 2]
            if c == 0.0:
                continue
            q = (a + dy) % 2
            d = (a + dy - q) // 2
            coff = b + dx + 2
            bands = groups.setdefault((q, coff), {})
            bands[d] = bands.get(d, 0.0) + c
    return groups


def _bmat(bands):
    B = np.zeros((KPART, MBLK), np.float32)
    for mm in range(MBLK):
        for d, c in bands.items():
            B[mm + 1 + d, mm] = c
    return B


TAILM = QH - 4 * MBLK       # quad rows in the per-image tail (8)
TAILP1 = TAILM + 2          # img1 tail partition base in the merged block


def _bmat_tail(bands):
    """Merged-tail band matrix: outputs 0..TAILM-1 are image 0's last quad
    rows (partitions mm+1+d), outputs TAILM..2*TAILM-1 are image 1's
    (partitions TAILP1 + (mm-TAILM) + 1 + d)."""
    B = np.zeros((KPART, MBLK), np.float32)
    for mm in range(2 * TAILM):
        base = (mm + 1) if mm < TAILM else (TAILP1 + mm - TAILM + 1)
        for d, c in bands.items():
            B[base + d, mm] = c
    return B


def _build_plan(index):
    """index: (3,2,2).  Returns (planes, pairs, bmats).

    planes: [(c, a, b, glist)]; each glist entry is (bmat_idx, src) where
    src = ("e", q, coff) reads the padded mosaic directly, or ("p", pi)
    reads pre-summed pair tile pi.  Groups of one plane whose band
    structures are identical (the symmetric column taps of the Malvar
    kernels) are folded into ONE matmul against a DVE-precomputed pair sum
    x[.,c_lo::2] + x[.,c_hi::2] -- pairs lists the distinct (q, c_lo,
    c_hi) tiles.  Identity planes are single-band matmuls.  For every band
    structure two matrices are interned: the main-block one and the
    merged-tail one (index + n_struct)."""
    bmain = []
    btail = []
    bkey = {}

    def intern(bands):
        key = tuple(sorted((d, round(v * 16)) for d, v in bands.items()))
        if key not in bkey:
            bkey[key] = len(bmain)
            bmain.append(_bmat(bands))
            btail.append(_bmat_tail(bands))
        return bkey[key]

    pairs = []
    pair_key = {}

    def intern_pair(q, lo, hi):
        key = (q, lo, hi)
        if key not in pair_key:
            pair_key[key] = len(pairs)
            pairs.append(key)
        return pair_key[key]

    # Symmetric-tap pairing (folding two same-band matmuls into one against
    # a DVE pair sum) measured SLOWER on HW: the strided fp16 tensor_tensor
    # runs at ~1.5us/tile, overloading DVE for a ~4us/block PE saving.
    use_pairs = False

    planes = []
    for c in range(3):
        for a in range(2):
            for b in range(2):
                k = int(index[c, a, b])
                if k == 4:
                    glist = [(intern({0: 1.0}), ("e", a, b + 2))]
                    planes.append((c, a, b, glist))
                    continue
                groups = _matmul_groups(k, a, b)
                bysig = {}
                for (q, coff), bands in sorted(groups.items()):
                    sig = (q, tuple(sorted(
                        (d, round(v * 16)) for d, v in bands.items())))
                    bysig.setdefault(sig, []).append(coff)
                glist = []
                for (q, _sig), coffs in sorted(bysig.items()):
                    bands = groups[(q, coffs[0])]
                    while use_pairs and len(coffs) >= 2:
                        lo = coffs.pop(0)
                        hi = coffs.pop(-1)
                        glist.append(
                            (intern(bands), ("p", intern_pair(q, lo, hi)))
                        )
                    for coff in coffs:
                        glist.append((intern(bands), ("e", q, coff)))
                planes.append((c, a, b, glist))
    return planes, pairs, np.stack(bmain + btail)


# ------------------------------------------------------------ bass program
def build_nc(planes, pairs, n_bmats, reps=1):
    f32, f16 = mybir.dt.float32, mybir.dt.float16
    nc = bacc.Bacc("TRN2", target_bir_lowering=False, debug=False)
    x_d = nc.dram_tensor("x", [IMGS_PER_CORE, QH, 2, W], f16, kind="ExternalInput")
    bm_d = nc.dram_tensor("bm", [KPART, n_bmats, MBLK], f16, kind="ExternalInput")
    y_d = nc.dram_tensor(
        "y", [IMGS_PER_CORE, 3, QH, 2, W], f32, kind="ExternalOutput"
    )

    n_struct = n_bmats // 2  # main matrices; tail copies live at +n_struct
    i0s = [i * MBLK for i in range(QH // MBLK)]  # full main blocks
    i0_tail = len(i0s) * MBLK

    with tile.TileContext(nc) as tc:
        with (
            tc.tile_pool(name="consts", bufs=1) as cpool,
            tc.tile_pool(name="esrc", bufs=3) as epool,
            tc.tile_pool(name="pairs", bufs=2) as ptpool,
            tc.tile_pool(name="stage", bufs=3) as spool,
            tc.tile_pool(name="psum", bufs=8, space="PSUM") as ppool,
        ):
            bw = cpool.tile([KPART, n_bmats, MBLK], f16, tag="bw", name="bw")
            nc.scalar.dma_start(bw[:, :, :], bm_d[:, :, :])

            # Tail-block source rows live in their own long-lived tile; the
            # loads are issued mid-program (see gbi==5 below) so the data is
            # resident well before the final block -- a late dispatch sits
            # behind megabytes of queued stores and opens a PE gap (HAM
            # re-throttle) right before the tail.
            ter = cpool.tile([KPART, 2, W], f16, tag="ter", name="ter")
            # tail matmuls pad the contraction to all 128 partitions to keep
            # PE activity high (HAM); zero the unused rows so the zero
            # weights never multiply NaN garbage
            nc.gpsimd.memset(ter[:, :, :], 0)

            for rep in range(reps):
                # blocks: (kind, img, i0); merged tail handles both images.
                # Tail goes LAST: its matmuls cover the big final-block
                # store drain.
                blocks = [
                    ("main", img, i0)
                    for img in range(IMGS_PER_CORE)
                    for i0 in i0s
                ] + [("tail", 0, i0_tail)]

                for gbi, (kind, img, i0) in enumerate(blocks):
                    if kind == "main":
                        m, kblk = MBLK, MBLK + 2
                    else:
                        m, kblk = 2 * TAILM, TAILP1 + TAILM + 2

                    if gbi == 5:
                        # issue the tail-block loads here: queues are past
                        # the cold window and ~30us of slack remains
                        for im in range(IMGS_PER_CORE):
                            pb = im * TAILP1
                            tld = nc.sync if im == 0 else nc.scalar
                            for q in range(2):
                                tld.dma_start(
                                    ter[pb + TAILM + 1 : pb + TAILM + 2,
                                        q, :],
                                    x_d[im, QH - 1 : QH, 1:2, :],
                                )
                            tld.dma_start(
                                ter[pb : pb + TAILM + 1, :, :],
                                x_d[im, i0_tail - 1 : QH, :, :],
                            )

                    if kind == "main":
                        er = epool.tile([KPART, 2, W], f16, tag="er",
                                        name=f"er_{gbi}")
                        ld = nc.sync if gbi % 2 == 0 else nc.scalar
                        p_lo = 1 if i0 == 0 else 0
                        if i0 == 0:  # halo above: quad -1 -> row 0 twice
                            for q in range(2):
                                ld.dma_start(
                                    er[0:1, q, :], x_d[img, 0:1, 0:1, :]
                                )
                        if gbi == 0 or i0 == 0:
                            # first load of each image: split by partition
                            # range across both queues to halve delivery
                            # time (cold DMA window / image-boundary restart)
                            p_mid = (p_lo + kblk) // 2
                            nc.sync.dma_start(
                                er[p_lo : p_mid, :, :],
                                x_d[img, i0 - 1 + p_lo : i0 - 1 + p_mid, :, :],
                            )
                            nc.scalar.dma_start(
                                er[p_mid : kblk, :, :],
                                x_d[img, i0 - 1 + p_mid : i0 - 1 + kblk, :, :],
                            )
                        else:
                            ld.dma_start(
                                er[p_lo : kblk, :, :],
                                x_d[img, i0 - 1 + p_lo : i0 - 1 + kblk, :, :],
                            )
                    else:
                        er = ter  # preloaded at program start
                    e = epool.tile([KPART, 2, EW], f16, tag="e",
                                   name=f"e_{gbi}")
                    for q in range(2):
                        nc.vector.tensor_copy(
                            e[0:kblk, q, 2 : 2 + W], er[0:kblk, q, :]
                        )
                    # horizontal replication pad (tiny copies on idle GpSimd)
                    for q in range(2):
                        nc.gpsimd.tensor_copy(e[:, q, 0:1], e[:, q, 2:3])
                        nc.gpsimd.tensor_copy(e[:, q, 1:2], e[:, q, 2:3])
                        nc.gpsimd.tensor_copy(
                            e[:, q, W + 2 : W + 3], e[:, q, W + 1 : W + 2]
                        )
                        nc.gpsimd.tensor_copy(
                            e[:, q, W + 3 : W + 4], e[:, q, W + 1 : W + 2]
                        )

                    # symmetric-tap pair sums (DVE): one tile per distinct
                    # (q, c_lo, c_hi); folds two matmuls into one
                    pt = {}
                    for pi_, (q, lo, hi) in enumerate(pairs):
                        t = ptpool.tile([KPART, QW], f16, tag=f"pt{pi_}",
                                        name=f"pt{pi_}_{gbi}")
                        nc.vector.tensor_tensor(
                            t[0:kblk, :],
                            e[0:kblk, q, lo : lo + W - 1 : 2],
                            e[0:kblk, q, hi : hi + W - 1 : 2],
                            mybir.AluOpType.add,
                        )
                        pt[pi_] = t

                    stg = {}
                    for c in range(3):
                        stg[c] = spool.tile([MBLK, 2, W], f32, tag=f"st{c}",
                                            name=f"st{c}_{gbi}")

                    boff = 0 if kind == "main" else n_struct
                    # tail matmuls are padded to the full 126x128 shape
                    # (extra rows/columns have all-zero weights): narrow
                    # matmuls read as low PE activity and trip the HAM
                    # clock-gate down to 1.2 GHz
                    m_mm = MBLK
                    kblk_mm = KPART if kind == "tail" else kblk
                    for pi, (c, a, b, glist) in enumerate(planes):
                        ps = ppool.tile([MBLK, QW], f32, tag="ps",
                                        name=f"ps{c}{a}{b}_{gbi}")
                        nmm = len(glist)
                        for i_mm, (bmi, src) in enumerate(glist):
                            if src[0] == "e":
                                rhs = e[0:kblk_mm, src[1],
                                        src[2] : src[2] + W - 1 : 2]
                            else:
                                rhs = pt[src[1]][0:kblk_mm, :]
                            nc.tensor.matmul(
                                ps[0:m_mm, :],
                                bw[0:kblk_mm, boff + bmi, 0:m_mm],
                                rhs,
                                start=(i_mm == 0),
                                stop=(i_mm == nmm - 1),
                            )
                        dst = stg[c][0:m, a, b : b + W - 1 : 2]
                        if pi % 2 == 0:
                            nc.vector.tensor_copy(dst, ps[0:m, :])
                        else:
                            nc.scalar.copy(dst, ps[0:m, :])

                    for c in range(3):
                        eng = nc.sync if (gbi + c) % 2 == 1 else nc.scalar
                        if kind == "main":
                            eng.dma_start(
                                y_d[img, c, i0 : i0 + m, :, :],
                                stg[c][0:m, :, :],
                            )
                        else:
                            for im in range(IMGS_PER_CORE):
                                eng.dma_start(
                                    y_d[im, c, i0:QH, :, :],
                                    stg[c][im * TAILM : (im + 1) * TAILM, :, :],
                                )
    nc.compile()
    return nc


# ------------------------------------------------------------- SPMD runner
_CACHE = {}


def _get_compiled(index_key, planes, pairs, n_bmats, reps=1):
    key = (index_key, reps)
    if key not in _CACHE:
        _CACHE[key] = build_nc(planes, pairs, n_bmats, reps=reps)
    return _CACHE[key]


_RUNNER_CACHE = {}


def make_runner(nc, n_cores=N_CORES):
    """Cached jitted SPMD executor mirroring bass2jax.run_bass_via_pjrt's
    multi-core path, reusable across calls without re-tracing."""
    import jax
    import concourse.mybir as mybir_
    from concourse import bass2jax
    from jax.experimental.shard_map import shard_map
    from jax.sharding import Mesh, PartitionSpec

    bass2jax.install_neuronx_cc_hook()

    partition_name = (
        nc.partition_id_tensor.name if nc.partition_id_tensor else None
    )
    in_names, out_names, out_avals, zero_outs = [], [], [], []
    for alloc in nc.m.functions[0].allocations:
        if not isinstance(alloc, mybir_.MemoryLocationSet):
            continue
        name = alloc.memorylocations[0].name
        if alloc.kind == "ExternalInput":
            if name != partition_name:
                in_names.append(name)
        elif alloc.kind == "ExternalOutput":
            shape = tuple(alloc.tensor_shape)
            dtype = mybir_.dt.np(alloc.dtype)
            out_names.append(name)
            out_avals.append(jax.core.ShapedArray(shape, dtype))
            zero_outs.append(np.zeros(shape, dtype))
    n_params = len(in_names)
    n_outs = len(out_avals)
    all_in_names = in_names + out_names
    if partition_name is not None:
        all_in_names.append(partition_name)

    def _body(*args):
        operands = list(args)
        if partition_name is not None:
            operands.append(bass2jax.partition_id_tensor())
        outs = bass2jax._bass_exec_p.bind(
            *operands,
            out_avals=tuple(out_avals),
            in_names=tuple(all_in_names),
            out_names=tuple(out_names),
            lowering_input_output_aliases=(),
            sim_require_finite=True,
            sim_require_nnan=True,
            nc=nc,
        )
        return tuple(outs)

    devices = jax.devices()[:n_cores]
    mesh = Mesh(np.asarray(devices), ("core",))
    sharded = jax.jit(
        shard_map(
            _body, mesh=mesh,
            in_specs=(PartitionSpec("core"),) * (n_params + n_outs),
            out_specs=(PartitionSpec("core"),) * n_outs,
            check_rep=False,
        ),
        donate_argnums=tuple(range(n_params, n_params + n_outs)),
        keep_unused=True,
    )

    # Output buffers are donated scratch: create them ON DEVICE (sharded)
    # instead of uploading host zeros -- uploading 24 MiB/core of zeros
    # overlaps NEFF execution and slows every kernel DMA at startup.
    import jax.numpy as jnp
    from jax.sharding import NamedSharding

    zero_shardings = tuple(
        NamedSharding(mesh, PartitionSpec("core")) for _ in zero_outs
    )
    device_zeros = jax.jit(
        lambda: tuple(
            jnp.zeros((n_cores * z.shape[0], *z.shape[1:]), z.dtype)
            for z in zero_outs
        ),
        out_shardings=zero_shardings,
    )

    def run(in_maps):
        concat_in = [
            np.concatenate([np.asarray(m[name]) for m in in_maps], axis=0)
            for name in in_names
        ]
        out_arrs = sharded(*concat_in, *device_zeros())
        return [
            {
                name: np.asarray(out_arrs[i]).reshape(
                    n_cores, *out_avals[i].shape
                )[c]
                for i, name in enumerate(out_names)
            }
            for c in range(n_cores)
        ]

    return run


def get_runner(reps=1, index=None, **_ignored):
    if index is None:
        index = INDICES_RGGB
    index3 = np.asarray(index).reshape(3, 2, 2)
    ikey = tuple(index3.flatten().tolist())
    key = (ikey, reps)
    if key not in _RUNNER_CACHE:
        planes, pairs, bmats = _build_plan(index3)
        nc = _get_compiled(ikey, planes, pairs, len(bmats), reps=reps)
        run = make_runner(nc)
        _RUNNER_CACHE[key] = (run, bmats)
    return _RUNNER_CACHE[key]


def pack_bmats(bmats):
    # [nb, KPART, MBLK] -> [KPART, nb, MBLK] contiguous fp16
    return np.ascontiguousarray(
        np.transpose(bmats, (1, 0, 2)).astype(np.float16)
    )


def make_inmaps(x, bmats):
    n = x.shape[0]
    xs = np.ascontiguousarray(x.reshape(n, QH, 2, W), dtype=np.float16)
    per = n // N_CORES
    bm = pack_bmats(bmats)
    return [
        {"x": xs[c * per : (c + 1) * per], "bm": bm}
        for c in range(N_CORES)
    ]


def kernel(x, bayer_pattern):
    x = np.ascontiguousarray(np.asarray(x), dtype=np.float32)
    bp = np.asarray(bayer_pattern)
    assert bp.reshape(-1, 4).shape[0] == 1, "per-batch bayer patterns unsupported"
    index = _calc_index(bp)
    run, bmats = get_runner(index=index)
    results = run(make_inmaps(x.reshape(x.shape[0], H, W), bmats))
    out = np.concatenate(
        [r["y"].reshape(IMGS_PER_CORE, 3, H, W) for r in results], axis=0
    )
    return out.astype(np.float32)
